# revision 5
# baseline (speedup 1.0000x reference)
"""Trainium2 Bass kernel for nn_GroupedKAAttention.

The reference network ends in ``jax.nn.softmax(attn, axis=-1)`` where
``attn`` has shape (B, 1, 1): the softmax normalizes over a singleton
axis, so the output is exactly 1.0 for every finite input — independent
of q, k and all weights (softmax(x) over one element is e^0 = 1 after
the max-subtraction). All inputs are finite randn fills, so the entire
MLP pipeline is dead code under constant folding; the mathematically
exact kernel writes ones.

Each of the 8 cores runs a one-instruction program: a single SP-issued
DMA that copies a 64-element block of ones (supplied as a tiny input)
into its slice of the (512,1,1) output, followed by the completion-
semaphore wait. Cost-model time ~2.2us, fully dominated by the fixed
DMA issue latency (HWDGE gen + DGE start delay + completion-semaphore
propagation).

The only non-obvious trick: Bass emits four const-pool memsets plus an
all-engine barrier at module init, which serializes ~200ns ahead of the
first user instruction. Nothing in this program reads the const pool or
crosses engines, so the init barrier is elided during construction
(restored immediately after), letting the SP engine issue the output
DMA at t~0.
"""

import os
import sys

import numpy as np

for _p in ("/opt/trn_rl_repo", "/root/.axon_site/_ro/trn_rl_repo"):
    if os.path.isdir(_p) and _p not in sys.path:
        sys.path.append(_p)

import concourse.bass as bass
import concourse.mybir as mybir
from concourse import bacc
from concourse import bass_utils

F32 = mybir.dt.float32

B = 512          # batch; output shape is (B, 1, 1)
NC = 8           # cores
BSLICE = B // NC  # 64 output elements per core

_CACHE = {}


def _build_program(skip_init_barrier=True):
    # Elide the init-time all-engine barrier: it only orders the const-pool
    # memsets (unused here) against user code, and costs ~200ns of serial
    # time before the first instruction. Restored right after construction
    # so collectives/blocks in any other program are unaffected.
    orig_barrier = bass.Bass.all_engine_barrier
    if skip_init_barrier:
        bass.Bass.all_engine_barrier = lambda self, **kw: None
    try:
        nc = bacc.Bacc("TRN2", target_bir_lowering=False, debug=False,
                       num_devices=NC)
    finally:
        bass.Bass.all_engine_barrier = orig_barrier

    ones_d = nc.dram_tensor("ones", [1, BSLICE], F32, kind="ExternalInput")
    out_d = nc.dram_tensor("out", [1, BSLICE], F32, kind="ExternalOutput")
    with nc.semaphore("dma_sem") as dma_sem:
        nc.sync.dma_start(out_d[:, :], ones_d[:, :]).then_inc(dma_sem, 16)
        nc.sync.wait_ge(dma_sem, 16)
    nc.compile()
    return nc


def _get_nc(skip_init_barrier=True):
    key = "nc" if skip_init_barrier else "nc_vanilla"
    if key not in _CACHE:
        _CACHE[key] = _build_program(skip_init_barrier)
    return _CACHE[key]


def _make_in_maps(**inputs):
    ones = np.ones((1, BSLICE), dtype=np.float32)
    return [{"ones": ones} for _ in range(NC)]


def _run(in_maps, trace=False, skip_init_barrier=True, **kwargs):
    nc = _get_nc(skip_init_barrier)
    return bass_utils.run_bass_kernel_spmd(
        nc, in_maps, core_ids=list(range(NC)), trace=trace, **kwargs
    )


def _gather(res):
    out = np.concatenate([r["out"][0] for r in res.results]).astype(np.float32)
    return out.reshape(B, 1, 1)


def kernel(**inputs):
    in_maps = _make_in_maps(**inputs)
    # The exact output is known (all-ones), so a failed device write is
    # detectable host-side. If the barrier-elided build misbehaves in this
    # environment, fall back to the vanilla build (init barrier intact).
    try:
        out = _gather(_run(in_maps, trace=False))
        if bool((out == 1.0).all()):
            return out
    except Exception:
        # One known raiser: BASS_TRACE=1 in an env whose axon client lacks
        # antenv.axon_hooks — run_bass_kernel_spmd crashes importing the NTFF
        # hook before executing anything. BASS_NEVER_TRACE forces the retry
        # down the working untraced path; envs with functional tracing never
        # reach this branch.
        os.environ.setdefault("BASS_NEVER_TRACE", "1")
    return _gather(_run(in_maps, trace=False, skip_init_barrier=False))
